# revision 11
# baseline (speedup 1.0000x reference)
"""Hanning template layer for TRN2: weighted sum of 4 Hanning correlations
== single 80-tap correlation.  out[b,j] = sum_i c[i] * x[b, j+i-40].

Device scheme (per core, 8 batch rows of L=65536, pure data parallel, fp16):
  Host pre-packs x (fp16) as xt_src[r*514 + 1 + n, k] = x_r[128n + k] with a
  zero 128-block before and after each row's 512 data blocks (halo).
  1. One xbar-transpose DMA loads XT[k, 514r + j] = xt_src[514r + j, k]
     -> blocked-transposed layout with zero halo columns, direct from HBM.
  2. Per row r, halo tile H (fp16, SBUF->SBUF DVE copies):
       H[q, n] = XT[q, 514r + n + 2]  for q in [0, 88)   (next-block taps)
       H[q, n] = XT[q, 514r + n]      for q in [88, 128) (prev-block taps)
     (rows 40..87 of Bh are zero, so H's middle stripe is don't-care but
      must be finite -> covered by the first copy.)
  3. Conv, natural-layout output, 2 matmuls per 128-col chunk k:
       pt[pp, 128k + m]  = sum_q XT[q, 514r + 1 + 4pp + k] * B1[q, m]
                         + sum_q  H[q, 4pp + k]            * Bh[q, m]
     where B1[q, m] = c[q - m + 40] (banded), Bh = corner triangles for the
     +-1 block shifts (disjoint row stripes, so they fold into one matrix).
     pt[pp, c] = y_r[512pp + c] -- already natural layout, no out-transpose.
  4. DVE/ACT copy PSUM->SBUF (cast fp16), one plain DMA out per pass.

Constraints baked in (learned on HW):
  - walrus codegen allows only ONE sync wait per instruction -> a post-pass
    splits residual multi-waits onto cloned per-engine Drain instructions.
"""

import copy as _copy

import numpy as np

import concourse.bass as bass
import concourse.mybir as mybir
from concourse.tile import TileContext
from concourse.bass_utils import run_bass_kernel_spmd

B, L = 64, 65536
N_CORES = 8
ROWS = B // N_CORES          # 8 rows per core
P = 128                      # partitions / block size
NBLK = L // P                # 512 blocks per row
WIN = NBLK + 2               # per-row window incl. zero halo cols
TAPS = 80
HALF = 40

F16 = mybir.dt.float16
F32 = mybir.dt.float32

WIDTHS = [10, 20, 30, 40]


def _combined_filter(template_weights: np.ndarray) -> np.ndarray:
    """softmax-weighted sum of hanning(2w) templates aligned at offset d=-40."""
    w = template_weights.astype(np.float64)
    e = np.exp(w - w.max())
    sm = e / e.sum()
    c = np.zeros(TAPS, dtype=np.float64)
    for t, wd in enumerate(WIDTHS):
        h = np.hanning(2 * wd)
        # contributes at filter index i = d + 40 for d in [-wd, wd)
        c[HALF - wd : HALF + wd] += sm[t] * h
    return c.astype(np.float32)


def _band_matrices(c: np.ndarray) -> np.ndarray:
    """Bs[s][k, m] = c[128(s-1) + k - m + 40] where in range, else 0."""
    Bs = np.zeros((3, P, P), dtype=np.float32)
    for s in range(3):
        off = P * (s - 1) + HALF
        for k in range(P):
            lo = max(0, k + off - (TAPS - 1))
            hi = min(P - 1, k + off)
            for m in range(lo, hi + 1):
                i = k - m + off
                if 0 <= i < TAPS:
                    Bs[s, k, m] = c[i]
    return Bs


def _split_excess_waits(nc, limit=1):
    """Move excess sync waits onto cloned same-engine Drain instructions
    (walrus codegen rejects >1 wait per instruction)."""
    drain_tmpl = {}
    for func in nc.m.functions:
        for bb in func.blocks:
            for inst in bb.instructions:
                if inst.opcode == "Drain" and inst.engine not in drain_tmpl:
                    drain_tmpl[inst.engine] = inst
    for func in nc.m.functions:
        for bb in func.blocks:
            changed = False
            out = []
            for inst in bb.instructions:
                si = inst.sync_info
                if si and len(si.on_wait) > limit:
                    waits = list(si.on_wait)
                    keep, extra = waits[-limit:], waits[:-limit]
                    tmpl = inst if inst.opcode == "Drain" else drain_tmpl.get(inst.engine)
                    assert tmpl is not None, (
                        f"no drain template for engine {inst.engine} ({inst.opcode})"
                    )
                    for j in range(0, len(extra), limit):
                        cln = _copy.deepcopy(tmpl)
                        cln.name = f"{inst.name}w{j}"
                        cln.engine = inst.engine
                        cln.sync_info = mybir.SyncInfo(
                            on_wait=extra[j : j + limit], on_update=[]
                        )
                        out.append(cln)
                        changed = True
                    si.on_wait = keep
                    inst.sync_info = si
                out.append(inst)
            if changed:
                bb.instructions = out


def emit_one_pass(nc, tc, pool, pp, xt_src, y, b1, bh):
    # 1. whole-shard load: XT[k, 514r + j] (fp16, 1.05 MB).  The host already
    # stores x transposed+blocked (partition-major), so this is a plain,
    # fully-contiguous DMA at line rate.
    xt = pool.tile([P, ROWS * WIN], F16, tag="xt")
    nc.sync.dma_start(out=xt, in_=xt_src[:, :])

    out_sb = pool.tile([P, ROWS * NBLK], F16, tag="out_sb")
    for r in range(ROWS):
        base = r * WIN
        # 2. halo tile (stripe copies; middle stripe harmless but finite)
        h = pool.tile([P, NBLK], F16, tag="h")
        nc.vector.tensor_copy(out=h[0:64, :], in_=xt[0:64, base + 2 : base + 2 + NBLK])
        nc.vector.tensor_copy(out=h[64:P, :], in_=xt[64:P, base : base + NBLK])

        # 3. conv: natural-layout output, 2 matmuls per 128-col chunk
        pt = pp.tile([P, NBLK], F32, tag="pt")
        for k in range(4):
            dst = pt[:, k * P : (k + 1) * P]
            span = 4 * (P - 1) + 1  # 128 strided columns
            nc.tensor.matmul(
                dst, xt[:, base + 1 + k : base + 1 + k + span : 4], b1,
                start=True, stop=False,
            )
            nc.tensor.matmul(
                dst, h[:, k : k + span : 4], bh,
                start=False, stop=True,
            )

        # 4. PSUM -> SBUF (cast fp16) on DVE (ACT copies are 2-9x slower)
        nc.vector.tensor_copy(out=out_sb[:, r * NBLK : (r + 1) * NBLK], in_=pt)

    # y_dev keeps the SBUF layout (host unpacks): fully-contiguous store
    nc.sync.dma_start(out=y[:, :], in_=out_sb[:, :])


def build_nc(reps: int = 1, unroll: int = 8):
    """Build the kernel IR.  reps=1 (default) is the production kernel.
    reps>1 wraps the whole pass in a hardware For_i loop executing the
    identical pass back-to-back `reps` times (used by test.py to amortize
    the axon dispatch overhead out of the HW-time measurement)."""
    nc = bass.Bass()
    xt_src = nc.dram_tensor("xt_src", [P, ROWS * WIN], F16, kind="ExternalInput")
    bmats = nc.dram_tensor("bmats", [P, 2 * P], F16, kind="ExternalInput")
    y = nc.dram_tensor("y", [P, ROWS * NBLK], F16, kind="ExternalOutput")

    with TileContext(nc) as tc:
        with (
            tc.tile_pool(name="sbuf", bufs=2) as pool,
            tc.tile_pool(name="cpool", bufs=1) as cpool,
            tc.tile_pool(name="psum", bufs=6, space="PSUM") as pp,
        ):
            bm = cpool.tile([P, 2 * P], F16)
            nc.sync.dma_start(out=bm, in_=bmats[:, :])
            b1 = bm[:, 0:P]
            bh = bm[:, P : 2 * P]

            def emit_pass():
                emit_one_pass(nc, tc, pool, pp, xt_src, y, b1, bh)

            if reps == 1:
                emit_pass()
            else:
                assert reps % unroll == 0
                with tc.For_i(0, reps // unroll, 1):
                    for _ in range(unroll):
                        emit_pass()

    _split_excess_waits(nc)
    return nc


def _host_consts(template_weights: np.ndarray) -> np.ndarray:
    c = _combined_filter(np.asarray(template_weights, dtype=np.float32))
    Bs = _band_matrices(c)
    bh = Bs[0] + Bs[2]
    # the two corner matrices live in disjoint row stripes (k>=88 / k<40)
    assert not (np.any(Bs[0][:88]) or np.any(Bs[2][40:])), "halo stripes overlap"
    return np.concatenate([Bs[1], bh], axis=1).astype(np.float16)


def _host_pack_x(x: np.ndarray) -> np.ndarray:
    """x [64, 65536] f32 -> per-core transposed+blocked fp16 [8, 128, 4112]:
    xh[c][k, 514r + 1 + n] = x[8c + r, 128n + k], zero halo cols at j=0, 513."""
    xb = x.astype(np.float16).reshape(N_CORES, ROWS, NBLK, P)
    packed = np.zeros((N_CORES, ROWS, WIN, P), dtype=np.float16)
    packed[:, :, 1 : NBLK + 1, :] = xb
    # [c, r, j, k] -> [c, k, r, j]
    return np.ascontiguousarray(packed.transpose(0, 3, 1, 2)).reshape(
        N_CORES, P, ROWS * WIN
    )


def _host_unpack_y(y_dev: np.ndarray) -> np.ndarray:
    """y_dev [8 cores, 128, 4096] fp16 (y_dev[c][p, r*512+cc] = y[8c+r][512p+cc])
    -> y [64, 65536] fp32."""
    y = y_dev.astype(np.float32).reshape(N_CORES, P, ROWS, NBLK)
    return np.ascontiguousarray(y.transpose(0, 2, 1, 3)).reshape(B, L)


_NC_CACHE = None


def kernel(x: np.ndarray, template_weights: np.ndarray) -> np.ndarray:
    global _NC_CACHE
    x = np.ascontiguousarray(np.asarray(x, dtype=np.float32))
    bmats = _host_consts(template_weights)
    xs = _host_pack_x(x)

    if _NC_CACHE is None:
        _NC_CACHE = build_nc()
    nc = _NC_CACHE

    in_maps = [
        {"xt_src": xs[core], "bmats": bmats} for core in range(N_CORES)
    ]
    res = run_bass_kernel_spmd(nc, in_maps, core_ids=list(range(N_CORES)))
    return _host_unpack_y(np.stack([r["y"] for r in res.results], axis=0))


# revision 12
# speedup vs baseline: 2.1221x; 2.1221x over previous
"""Hanning template layer for TRN2: weighted sum of 4 Hanning correlations
== single 80-tap correlation.  out[b,j] = sum_i c[i] * x[b, j+i-40].

Device scheme (per core, 8 batch rows of L=65536, pure data parallel, fp16):

  Host packs x fp16, transposed + blocked with the blocks aligned to the
  conv-window START (j-40), one zero-padded halo block at the end:
      xt2[k, 513r + n] = x_r[128n + k - 40]   (n in [0, 512], zeros outside)
  Because the 208-sample input window of a 128-output block spans exactly two
  such blocks, the conv is exactly TWO matmuls per output chunk -- no halo
  tile, no stripe copies, no memsets:

      pt[pp, 128g + m] = sum_q xt2[q, w + 4pp] C0[q, m]
                       + sum_q xt2[q, w + 4pp + 1] C1[q, m],   w = 513r + g
      C0[q, m] = c[q - m],  C1[q, m] = c[128 + q - m]  (banded, host-built)

  pt[pp, c] = y_r[512pp + c] is already natural layout; PSUM->SBUF copies are
  two big ops (4 rows each) split DVE/ACT (DVE ops pay a ~0.3-0.5us pipeline
  drain each -- minimize op COUNT); y stays in SBUF layout in HBM (fully
  contiguous store) and the host unpacks.

Constraints baked in (learned on HW):
  - walrus codegen allows only ONE sync wait per instruction -> a post-pass
    splits residual multi-waits onto cloned per-engine Drain instructions.
  - engines address partition ranges starting only at multiples of 32.
  - DMA xbar-transpose is ~6us/MB slower than plain DMA: transpose on HOST.
"""

import copy as _copy

import numpy as np

import concourse.bass as bass
import concourse.mybir as mybir
from concourse.tile import TileContext
from concourse.bass_utils import run_bass_kernel_spmd

B, L = 64, 65536
N_CORES = 8
ROWS = B // N_CORES          # 8 rows per core
P = 128                      # partitions / block size
NBLK = L // P                # 512 output blocks per row
WIN = NBLK + 1               # input blocks per row (one tail halo block)
TAPS = 80
HALF = 40
RG = 4                       # rows per PSUM tile group (4 banks)

F16 = mybir.dt.float16
F32 = mybir.dt.float32

WIDTHS = [10, 20, 30, 40]


def _combined_filter(template_weights: np.ndarray) -> np.ndarray:
    """softmax-weighted sum of hanning(2w) templates aligned at offset d=-40."""
    w = template_weights.astype(np.float64)
    e = np.exp(w - w.max())
    sm = e / e.sum()
    c = np.zeros(TAPS, dtype=np.float64)
    for t, wd in enumerate(WIDTHS):
        h = np.hanning(2 * wd)
        # contributes at filter index i = d + 40 for d in [-wd, wd)
        c[HALF - wd : HALF + wd] += sm[t] * h
    return c.astype(np.float32)


def _host_consts(template_weights: np.ndarray) -> np.ndarray:
    """[C0 | C1] fp16: C0[q, m] = c[q - m], C1[q, m] = c[128 + q - m]."""
    c = _combined_filter(np.asarray(template_weights, dtype=np.float32))
    q = np.arange(P)[:, None]
    m = np.arange(P)[None, :]
    idx0 = q - m
    idx1 = 128 + q - m
    cm = np.concatenate([c, np.zeros(1, np.float32)])  # index TAPS -> 0
    C0 = cm[np.clip(idx0, 0, TAPS)] * ((idx0 >= 0) & (idx0 < TAPS))
    C1 = cm[np.clip(idx1, 0, TAPS)] * ((idx1 >= 0) & (idx1 < TAPS))
    return np.concatenate([C0, C1], axis=1).astype(np.float16)


def _host_pack_x(x: np.ndarray) -> np.ndarray:
    """x [64, 65536] f32 -> per-core fp16 [8, 128, 8*513]:
    xh[c][k, 513r + n] = x[8c + r, 128n + k - 40] (zeros outside [0, L))."""
    x16 = x.astype(np.float16).reshape(N_CORES, ROWS, L)
    pad = np.zeros((N_CORES, ROWS, WIN * P), dtype=np.float16)
    pad[:, :, HALF : HALF + L] = x16
    blocks = pad.reshape(N_CORES, ROWS, WIN, P)      # [c, r, n, k]
    return np.ascontiguousarray(blocks.transpose(0, 3, 1, 2)).reshape(
        N_CORES, P, ROWS * WIN
    )


def _host_unpack_y(y_dev: np.ndarray) -> np.ndarray:
    """y_dev [8 cores, 128, 4096] fp16 (y_dev[c][p, r*512+cc] = y[8c+r][512p+cc])
    -> y [64, 65536] fp32."""
    y = y_dev.astype(np.float32).reshape(N_CORES, P, ROWS, NBLK)
    return np.ascontiguousarray(y.transpose(0, 2, 1, 3)).reshape(B, L)


def _split_excess_waits(nc, limit=1):
    """Move excess sync waits onto cloned same-engine Drain instructions
    (walrus codegen rejects >1 wait per instruction)."""
    drain_tmpl = {}
    for func in nc.m.functions:
        for bb in func.blocks:
            for inst in bb.instructions:
                if inst.opcode == "Drain" and inst.engine not in drain_tmpl:
                    drain_tmpl[inst.engine] = inst
    for func in nc.m.functions:
        for bb in func.blocks:
            changed = False
            out = []
            for inst in bb.instructions:
                si = inst.sync_info
                if si and len(si.on_wait) > limit:
                    waits = list(si.on_wait)
                    keep, extra = waits[-limit:], waits[:-limit]
                    tmpl = inst if inst.opcode == "Drain" else drain_tmpl.get(inst.engine)
                    assert tmpl is not None, (
                        f"no drain template for engine {inst.engine} ({inst.opcode})"
                    )
                    for j in range(0, len(extra), limit):
                        cln = _copy.deepcopy(tmpl)
                        cln.name = f"{inst.name}w{j}"
                        cln.engine = inst.engine
                        cln.sync_info = mybir.SyncInfo(
                            on_wait=extra[j : j + limit], on_update=[]
                        )
                        out.append(cln)
                        changed = True
                    si.on_wait = keep
                    inst.sync_info = si
                out.append(inst)
            if changed:
                bb.instructions = out


def emit_one_pass(nc, tc, pool, pp, xt_src, y, c0, c1):
    # 1. whole-shard load (host pre-transposed): plain fully-contiguous DMA
    xt = pool.tile([P, ROWS * WIN], F16, tag="xt")
    nc.sync.dma_start(out=xt, in_=xt_src[:, :])

    out_sb = pool.tile([P, ROWS * NBLK], F16, tag="out_sb")
    span = 4 * (P - 1) + 1  # 128 stride-4 columns
    for rg in range(ROWS // RG):
        # 2. conv: 2 matmuls per 128-col chunk, natural-layout PSUM output
        pt = pp.tile([P, RG * NBLK], F32, tag="pt")  # 4 rows, 4 PSUM banks
        for ro in range(RG):
            r = rg * RG + ro
            base = r * WIN
            for g in range(4):
                dst = pt[:, ro * NBLK + g * P : ro * NBLK + (g + 1) * P]
                w = base + g
                nc.tensor.matmul(
                    dst, xt[:, w : w + span : 4], c0, start=True, stop=False
                )
                nc.tensor.matmul(
                    dst, xt[:, w + 1 : w + 1 + span : 4], c1,
                    start=False, stop=True,
                )
        # 3. one big PSUM -> SBUF copy (cast fp16) per 4 rows; DVE | ACT
        dst = out_sb[:, rg * RG * NBLK : (rg + 1) * RG * NBLK]
        if rg % 2 == 0:
            nc.vector.tensor_copy(out=dst, in_=pt)
        else:
            nc.scalar.copy(out=dst, in_=pt)

    # 4. y keeps the SBUF layout in HBM (host unpacks): contiguous store
    nc.sync.dma_start(out=y[:, :], in_=out_sb[:, :])


def build_nc(reps: int = 1, unroll: int = 8):
    """Build the kernel IR.  reps=1 (default) is the production kernel.
    reps>1 wraps the whole pass in a hardware For_i loop executing the
    identical pass back-to-back `reps` times (used by test.py to amortize
    the axon dispatch overhead out of the HW-time measurement)."""
    nc = bass.Bass()
    xt_src = nc.dram_tensor("xt_src", [P, ROWS * WIN], F16, kind="ExternalInput")
    bmats = nc.dram_tensor("bmats", [P, 2 * P], F16, kind="ExternalInput")
    y = nc.dram_tensor("y", [P, ROWS * NBLK], F16, kind="ExternalOutput")

    with TileContext(nc) as tc:
        with (
            tc.tile_pool(name="sbuf", bufs=2) as pool,
            tc.tile_pool(name="cpool", bufs=1) as cpool,
            tc.tile_pool(name="psum", bufs=2, space="PSUM") as pp,
        ):
            bm = cpool.tile([P, 2 * P], F16)
            nc.sync.dma_start(out=bm, in_=bmats[:, :])
            c0 = bm[:, 0:P]
            c1 = bm[:, P : 2 * P]

            def emit_pass():
                emit_one_pass(nc, tc, pool, pp, xt_src, y, c0, c1)

            if reps == 1:
                emit_pass()
            else:
                assert reps % unroll == 0
                with tc.For_i(0, reps // unroll, 1):
                    for _ in range(unroll):
                        emit_pass()

    _split_excess_waits(nc)
    return nc


_NC_CACHE = None


def kernel(x: np.ndarray, template_weights: np.ndarray) -> np.ndarray:
    global _NC_CACHE
    x = np.ascontiguousarray(np.asarray(x, dtype=np.float32))
    bmats = _host_consts(template_weights)
    xs = _host_pack_x(x)

    if _NC_CACHE is None:
        _NC_CACHE = build_nc()
    nc = _NC_CACHE

    in_maps = [
        {"xt_src": xs[core], "bmats": bmats} for core in range(N_CORES)
    ]
    res = run_bass_kernel_spmd(nc, in_maps, core_ids=list(range(N_CORES)))
    return _host_unpack_y(np.stack([r["y"] for r in res.results], axis=0))
